# revision 46
# baseline (speedup 1.0000x reference)
"""Multi-head attention (B=8, N=1024, C=768, H=12) on 8 TRN2 NeuronCores.

Data-parallel over batch: core b computes batch element b end-to-end.

v3: software-pipelined stream over head PAIRS (row-packed 64-contraction
STs, exp(S*scale-5) on ScalarE, PV with the ones-column denominator
trick), plus:

  - ALL softmax-denominator reciprocals run on ScalarE as exp(-ln d)
    (Ln/Exp share one act table).  The old batched DVE reciprocals were
    ~8us each and serialized behind PV drains in the DVE queue, stalling
    the 2-slot ov PSUM ring and opening ~4us PE gaps.
  - every DMA issues from the SP (sync) HWDGE queue only: issuing from
    the Activation HWDGE queue in parallel throttled the whole chip ~9%
    (warm 512-row matmul 215->235ns) - measured, reproducible.
  - lead-in DMA order is consumption order (wqks j0, xr t2=0, xr t2=1,
    wqks jCC, ...), and a no-dep dummy Exp pulls the act-table load to
    ~9us so the first real EXP isn't stalled behind it.
  - pv10 trails the exp stream in the ov ring; pv11 trails in a 97th
    st-ring tile (allocated after all 96 ST allocations - earlier would
    poison the ring: a later ST tile would WAR-wait on pv11's late
    drain).  Their denominators land in split [2,N] tiles so the 8/9
    recip chain starts right at the last softmax EXP and 10/11's only
    waits on its own drain.
  - the output projection prefills c=0..1, then normalizes oT2/oT3
    (r_chunks moved out of the pair-5 chain - as chain items they formed
    a serial PE<->DVE chain through the ov ring that spilled past the
    stream end at cold p-state), then c=2..4 as recips resolve; only the
    c=5 accumulations, biases and ring-split output DMAs remain at the
    end.  Output stores are split 4/8-way across DMA rings (one 512KB
    dma_start lands on ~2 rings at ~22.5GB/s and drained ~11us past the
    last compute).

PSUM budget: ST ring 3 x [128,1024] (6 banks) + work ring 2 x [128,512]
(2 banks) = 8 banks.  Host side casts x/weights to fp16 and transposes
the fp32 outT result back.
"""

import os

import numpy as np

import concourse.bass as bass
import concourse.tile as tile
from concourse import mybir
from concourse.bass_utils import run_bass_kernel_spmd

B, N, C = 8, 1024, 768
H, D = 12, 64
NP = H // 2            # head pairs
SCALE = D ** -0.5
EXP_BIAS = -5.0
CC = C // 128          # 6 contraction chunks
NT2 = N // 512         # 2 free-dim chunks of 512
NT8 = N // 128         # 8 partition chunks of 128
F32 = mybir.dt.float32
F16 = mybir.dt.float16

_N_CORES = 8


def _split_multiwaits(nc, max_waits: int = 1):
    """The pinned walrus codegen supports one embedded sync-wait per engine
    instruction (single EVENTS slot in the TPB ISA).  Tile's tail drain /
    barriers accumulate several; hoist all-but-one wait onto same-engine
    NoOps placed immediately before the instruction (waits AND, so order is
    irrelevant)."""
    n_split = 0
    for f in nc.m.functions:
        for blk in f.blocks:
            insts = blk.instructions
            if not any(
                ins.sync_info is not None and len(ins.sync_info.on_wait) > max_waits
                for ins in insts
            ):
                continue
            new_list = []
            for ins in insts:
                si = ins.sync_info
                if si is not None and len(si.on_wait) > max_waits:
                    waits = list(si.on_wait)
                    hoist, keep = waits[:-max_waits], waits[-max_waits:]
                    for w in hoist:
                        nop = mybir.InstNoOp(name=nc.get_next_instruction_name())
                        nop.engine = ins.engine
                        nop.sync_info = mybir.SyncInfo(on_wait=[w], on_update=[])
                        new_list.append(nop)
                        n_split += 1
                    ins.sync_info = mybir.SyncInfo(
                        on_wait=keep, on_update=list(si.on_update)
                    )
                new_list.append(ins)
            blk.instructions = new_list
    return n_split


def _hoist_st_waits(nc):
    """ST (64-contraction) matmuls packed at tile_position (0,0)/(64,0) can
    execute CONCURRENTLY on the PE's two quadrant rows, but only when the
    trailing matmul reaches the array with no unsatisfied sync-wait (the
    trace shows fused trailers retiring in ~3ns; any wait serializes them at
    full cost).  The st-ring WAR waits land on the first writer of each
    recycled PSUM tile, i.e. in the middle of the 4-matmul ST group.  Hoist
    every wait inside a run of consecutive ST units (Ldweights[64p] +
    NoOps + Matmult) onto fresh PE NoOps placed just before the run, so the
    run's matmuls issue back-to-back and pair up.  Waits only move EARLIER
    on the same engine queue, which is always safe (their producers never
    depend on the skipped instructions)."""
    n_runs = 0
    for f in nc.m.functions:
        for blk in f.blocks:
            insts = blk.instructions
            # collect (block-pos) of PE instructions
            pe_pos = [i for i, ins in enumerate(insts)
                      if ins.engine == mybir.EngineType.PE]

            def is_st_ldw(ins):
                if type(ins).__name__ != "InstLdweights":
                    return False
                try:
                    ap = list(ins.ins[0].ap)
                    return ap[0][1] == 64
                except Exception:
                    return False

            # walk the PE subsequence, grouping ST units into runs.  A run
            # window may cover at most 2 distinct output PSUM tiles (= one
            # ST pair): with the 3-deep st ring, a longer window can contain
            # the producer matmuls of the very ACT a hoisted WAR wait blocks
            # on (ring wrap) -> PE/ScalarE deadlock.  Windows of <=2 newest
            # allocations can never include the tile an in-window WAR wait's
            # ACT reads (that tile is >=3 allocations older).
            def mm_out_ref(ins):
                try:
                    return ins.outs[0].memref
                except Exception:
                    return None

            runs = []  # list of lists of block positions
            cur = []
            cur_tiles = []

            def flush():
                nonlocal cur, cur_tiles
                if len(cur) >= 2:
                    runs.append(cur)
                cur = []
                cur_tiles = []

            k = 0
            while k < len(pe_pos):
                ins = insts[pe_pos[k]]
                if is_st_ldw(ins):
                    unit = [pe_pos[k]]
                    k += 1
                    # absorb NoOps then the matmult
                    while k < len(pe_pos) and type(insts[pe_pos[k]]).__name__ == "InstNoOp":
                        unit.append(pe_pos[k])
                        k += 1
                    if k < len(pe_pos) and type(insts[pe_pos[k]]).__name__ == "InstMatmult":
                        mm = insts[pe_pos[k]]
                        unit.append(pe_pos[k])
                        k += 1
                        ref = mm_out_ref(mm)
                        if ref not in cur_tiles and len(cur_tiles) >= 2:
                            flush()
                        if ref not in cur_tiles:
                            cur_tiles.append(ref)
                        cur.append(unit)
                        continue
                    flush()
                else:
                    flush()
                    k += 1
            flush()

            if not runs:
                continue
            # hoist waits: build new instruction list in one pass
            prepend = {}  # head block-pos -> list of waits
            for run in runs:
                waits = []
                for unit in run:
                    for pos in unit:
                        si = insts[pos].sync_info
                        if si is not None and si.on_wait:
                            waits.extend(si.on_wait)
                            insts[pos].sync_info = mybir.SyncInfo(
                                on_wait=[], on_update=list(si.on_update)
                            )
                if waits:
                    prepend[run[0][0]] = waits
                    n_runs += 1
            if not prepend:
                continue
            new_list = []
            for i, ins in enumerate(insts):
                if i in prepend:
                    for w in prepend[i]:
                        nop = mybir.InstNoOp(name=nc.get_next_instruction_name())
                        nop.engine = mybir.EngineType.PE
                        nop.sync_info = mybir.SyncInfo(on_wait=[w], on_update=[])
                        new_list.append(nop)
                new_list.append(ins)
            blk.instructions = new_list
    return n_runs


def _dedup_st_ldw(nc):
    """Drop redundant ST Ldweights: within a burst the same 64-row
    stationary (quadrant row 0-63 or 64-127) is re-loaded for the second
    free-dim half although no other load touched that quadrant.  Replacing
    the reload with a NoOp (keeping its sync_info so producer/consumer sems
    still fire) removes exposed load time and leaves matmuls back-to-back,
    which the PE can chain.  Any non-64-partition Ldweights clobbers both
    quadrants conservatively."""
    n_drop = 0
    for f in nc.m.functions:
        for blk in f.blocks:
            last_q = {0: None, 64: None}
            insts = blk.instructions
            for i, ins in enumerate(insts):
                if ins.engine != mybir.EngineType.PE:
                    continue
                if type(ins).__name__ != "InstLdweights":
                    continue
                try:
                    pap = ins.ins[0]
                    ap = list(pap.ap)
                    pstride, pnum = ap[0]
                    off = pap.offset
                    memref = pap.memref
                except Exception:
                    last_q = {0: None, 64: None}
                    continue
                if pnum != 64:
                    last_q = {0: None, 64: None}
                    continue
                base = off // pstride if pstride else 0
                q = 0 if base < 64 else 64
                key = (memref, off)
                if last_q[q] == key:
                    nop = mybir.InstNoOp(name=nc.get_next_instruction_name())
                    nop.engine = mybir.EngineType.PE
                    nop.sync_info = ins.sync_info
                    insts[i] = nop
                    n_drop += 1
                else:
                    last_q[q] = key
    return n_drop


def _build(split: bool = True):
    nc = bass.Bass()
    xT = nc.declare_dram_parameter("xT", [C, N], F16, isOutput=False)
    wqkT = nc.declare_dram_parameter("wqkT", [C, 2 * C], F16, isOutput=False)
    wvT = nc.declare_dram_parameter("wvT", [C, C], F16, isOutput=False)
    woT = nc.declare_dram_parameter("woT", [C, C], F16, isOutput=False)
    bo = nc.declare_dram_parameter("bo", [C, 1], F32, isOutput=False)
    ind4_d = nc.declare_dram_parameter("ind4", [4, 2 * 128], F16, isOutput=False)
    outT = nc.declare_dram_parameter("outT", [C, N], F32, isOutput=True)

    with tile.TileContext(nc) as tc:
        with (
            tc.tile_pool(name="sb", bufs=1) as sb,
            tc.tile_pool(name="ph2", bufs=34) as ph2,
            tc.tile_pool(name="ph2s", bufs=3) as ph2s,
            tc.tile_pool(name="psum", bufs=1, space="PSUM") as psum,
        ):
            qkT = [
                sb.tile([128, N], F16, tag=f"qkT{j}", name=f"qkT{j}")
                for j in range(2 * CC)
            ]
            v_sb = [
                sb.tile([128, H * (D + 1)], F16, tag=f"v{t}", name=f"v{t}")
                for t in range(NT8)
            ]
            oT = [sb.tile([128, N], F16, tag=f"oT{c}", name=f"oT{c}") for c in range(CC)]
            bo_t = [sb.tile([128, 1], F32, tag=f"bo{c}", name=f"bo{c}") for c in range(CC)]
            xr = [sb.tile([128, N], F16, tag=f"xr{c}", name=f"xr{c}") for c in range(CC)]
            wqk = [
                sb.tile([128, 2 * C], F16, tag=f"wqk{c}", name=f"wqk{c}")
                for c in range(CC)
            ]
            wv = [sb.tile([128, C], F16, tag=f"wv{c}", name=f"wv{c}") for c in range(CC)]
            wo = [sb.tile([128, C], F16, tag=f"wo{c}", name=f"wo{c}") for c in range(CC)]

            # DMA lead-in: the first ST needs xr[:, t2=0] + the wqks j=0/CC
            # duplicated slices.  Issue latency on one queue (~565ns per
            # dma_start on the SP sequencer) dominated the old 17us lead-in,
            # so spread the critical transfers across BOTH HWDGE queues:
            # sync takes x, scalar takes the small weight slices (the Act
            # engine is idle until the first EXP at ~26us).
            # NOTE: all DMAs must issue from the SP (sync) HWDGE queue.  Using
            # the Activation engine's HWDGE queue in parallel looked free on
            # paper but throttled the whole chip ~9% (every engine's warm
            # cycle time inflated 215->235ns/512rows) — measured +8.5us.
            wqks = {}
            _dma2 = nc.sync

            def wqks_dma(j, c, eng=None):
                js = slice(j * 128, (j + 1) * 128)
                sl = slice(c * 128, (c + 1) * 128)
                t = sb.tile([128, 128], F16, tag=f"wqks{j}_{c}", name=f"wqks{j}_{c}")
                (eng or nc.sync).dma_start(out=t, in_=wqkT[sl, js])
                wqks[(j, c)] = t

            # j0 slices via the Pool SWDGE queue: takes 6 issues off the
            # sync DGE's critical lead-in so the xr transfers start ~3.4us
            # earlier (sync DGE gen is ~565ns per dma_start, serial).
            for c in range(CC):
                wqks_dma(0, c, eng=nc.gpsimd)
            for c in range(CC):
                sl = slice(c * 128, (c + 1) * 128)
                nc.sync.dma_start(out=xr[c][:, 0:512], in_=xT[sl, 0:512])
            # xr t2=1 before wqks jCC: the lead-in order is 1a(0,0) [xr
            # t2=0 + wqks j0], 1a(0,1) [xr t2=1], 1a(CC,0) [wqks jCC]
            for c in range(CC):
                sl = slice(c * 128, (c + 1) * 128)
                nc.sync.dma_start(out=xr[c][:, 512:1024], in_=xT[sl, 512:1024])
            for c in range(CC):
                wqks_dma(CC, c)
            # pull the Exp act-table load off the first-EXP critical path:
            # a no-dep dummy exp right after the critical weight slices makes
            # the fixpoint table-load pass plant the load at ~9us instead of
            # right before the first real EXP.
            tbl = sb.tile([1, 2], F32, tag="tbl")
            nc.vector.memset(tbl[:, 0:1], 0.0)
            nc.scalar.activation(
                tbl[:, 1:2], tbl[:, 0:1], mybir.ActivationFunctionType.Exp
            )
            # (j=1 / CC+1 duplicates dropped: pair-1's 1a chain runs at
            # ~30-45us, after the full wqk tiles land (~20-25us) — emit_1a
            # falls back to wqk[c] slices; the 3KB freed feeds the exps pool)
            for c in range(CC):
                sl = slice(c * 128, (c + 1) * 128)
                nc.sync.dma_start(out=wv[c], in_=wvT[sl, :])
            for c in range(CC):
                sl = slice(c * 128, (c + 1) * 128)
                nc.sync.dma_start(out=wqk[c], in_=wqkT[sl, :])
            for c in range(CC):
                sl = slice(c * 128, (c + 1) * 128)
                nc.sync.dma_start(out=wo[c], in_=woT[sl, :])
                nc.sync.dma_start(out=bo_t[c], in_=bo[sl, :])

            ones12 = sb.tile([128, H], F16, tag="ones12")
            nc.vector.memset(ones12, 1.0)
            ebias = sb.tile([128, 1], F32, tag="ebias")
            nc.vector.memset(ebias, EXP_BIAS)

            dallA1 = sb.tile([4, N], F32, tag="dallA1")
            dallA2 = sb.tile([4, N], F32, tag="dallA2")
            # tail denominators alias rows 0-1 of the A tiles: lifetimes are
            # disjoint (lnA1/lnA2 consume the A rows around pair 3-4, the
            # B-group drains write at pair 5 / tail; Tile's WAR edges keep
            # it safe), freeing 8KB of columns for the exps pool.
            dallB89 = dallA1[0:2, :]
            dallB11 = dallA2[0:2, :]
            lnS = sb.tile([4, N], F32, tag="lnS")  # shared ln scratch (A+B)
            rec16A1 = sb.tile([4, N], F16, tag="rec16A1")
            rec16A2 = sb.tile([4, N], F16, tag="rec16A2")
            rec16B89 = sb.tile([2, N], F16, tag="rec16B89")
            rec16B11 = sb.tile([2, N], F16, tag="rec16B11")
            ind4_t = sb.tile([4, 2 * 128], F16, tag="ind4")
            nc.sync.dma_start(out=ind4_t, in_=ind4_d[:, :])

            def st_tile():
                return psum.tile([128, N], F32, tag="st", name="st", bufs=3)

            def ov_tile():
                return psum.tile([128, 512], F32, tag="ov", name="ov", bufs=2)

            # ---------------- emission helpers ----------------
            def emit_1a(j, t2):
                # qkT[j][:, t2-half] = sum_c wqk[c][:, j-block].T @ xr[c][:, t2]
                p = ov_tile()
                for c in range(CC):
                    w = (
                        wqks[(j, c)]
                        if (j, c) in wqks
                        else wqk[c][:, j * 128 : (j + 1) * 128]
                    )
                    nc.tensor.matmul(
                        p,
                        w,
                        xr[c][:, t2 * 512 : (t2 + 1) * 512],
                        start=(c == 0),
                        stop=(c == CC - 1),
                    )
                nc.vector.tensor_copy(qkT[j][:, t2 * 512 : (t2 + 1) * 512], p)

            def emit_1b(t8, nh):
                # v[t8][:, h*65:h*65+64] = sum_c xr[c][:, t8].T @ wv[c]
                p = ov_tile()
                for c in range(CC):
                    nc.tensor.matmul(
                        p[:, 0:384],
                        xr[c][:, t8 * 128 : (t8 + 1) * 128],
                        wv[c][:, nh * 384 : (nh + 1) * 384],
                        start=(c == 0),
                        stop=(c == CC - 1),
                    )
                v_view = v_sb[t8].rearrange("p (h e) -> p h e", e=D + 1)
                nc.vector.tensor_copy(
                    v_view[:, nh * 6 : (nh + 1) * 6, 0:D],
                    p[:, 0:384].rearrange("p (h d) -> p h d", d=D),
                )
                if nh == 1:
                    nc.vector.tensor_copy(
                        v_view[:, :, D : D + 1], ones12.unsqueeze(2)
                    )

            exps = {}

            def emit_st_pair(p_, kc):
                # row-packed: even head in PE rows 0-63, odd head in 64-127
                se = st_tile()
                so = st_tile()
                kb = slice(kc * 128, (kc + 1) * 128)
                for t2 in range(NT2):
                    sl = slice(t2 * 512, (t2 + 1) * 512)
                    nc.tensor.matmul(
                        se[:, sl], qkT[CC + p_][0:D, kb], qkT[p_][0:D, sl],
                        start=True, stop=True,
                    )
                    nc.tensor.matmul(
                        so[:, sl], qkT[CC + p_][D:128, kb], qkT[p_][D:128, sl],
                        start=True, stop=True,
                    )
                ee = ph2.tile([128, N], F16, tag="exps", name="exps")
                eo = ph2.tile([128, N], F16, tag="exps", name="exps")
                nc.scalar.activation(
                    ee, se, mybir.ActivationFunctionType.Exp, scale=SCALE, bias=ebias
                )
                nc.scalar.activation(
                    eo, so, mybir.ActivationFunctionType.Exp, scale=SCALE, bias=ebias
                )
                exps[(2 * p_, kc)] = ee
                exps[(2 * p_ + 1, kc)] = eo

            pv_state = {}

            def pv_mm_range(h, ovs, kcs):
                for kc in kcs:
                    e = exps.pop((h, kc))
                    for t2 in range(NT2):
                        nc.tensor.matmul(
                            ovs[t2][0 : D + 1, :],
                            v_sb[kc][:, h * (D + 1) : (h + 1) * (D + 1)],
                            e[:, t2 * 512 : (t2 + 1) * 512],
                            start=(kc == 0),
                            stop=(kc == NT8 - 1),
                        )

            def emit_pv_mms(h):
                ovs = [ov_tile() for _ in range(NT2)]
                pv_mm_range(h, ovs, range(NT8))
                return ovs

            def pv_q(h, q):
                # quarter-granularity PV chain items (4 MMs each) so a PV
                # burst never exceeds one slot's PE budget; q=0 allocates,
                # q=3 drains.  Parts of one head must stay adjacent in the
                # chain list (ov-ring WAR deadlock otherwise).
                if q == 0:
                    ovs = [ov_tile() for _ in range(NT2)]
                    pv_state[h] = ovs
                    pv_mm_range(h, ovs, range(2))
                else:
                    pv_mm_range(h, pv_state[h], range(2 * q, 2 * q + 2))
                    if q == 3:
                        emit_pv_drain(h, pv_state.pop(h))

            def ipv(h):
                return [lambda q=q: pv_q(h, q) for q in range(4)]

            def pv_open(h):
                # first half of the PV accumulation; holds both ov-ring
                # slots until pv_close — no other ov-allocating item may be
                # emitted in between (ring WAR would deadlock the PE FIFO)
                ovs = [ov_tile() for _ in range(NT2)]
                pv_state[h] = ovs
                pv_mm_range(h, ovs, range(NT8 // 2))

            def pv_close(h):
                ovs = pv_state.pop(h)
                pv_mm_range(h, ovs, range(NT8 // 2, NT8))
                emit_pv_drain(h, ovs)

            def emit_pv_drain(h, ovs, stg_first=False):
                po = (h % 2) * 64
                dtile, row = (
                    (dallA1, h) if h < 4
                    else (dallA2, h - 4) if h < 8
                    else (dallB89, h - 8) if h < 10
                    else (dallB11, h - 10)
                )
                for t2 in range(NT2):
                    o = ovs[t2]
                    qs = slice(t2 * 512, (t2 + 1) * 512)
                    if not stg_first:
                        nc.vector.tensor_copy(oT[h // 2][po : po + D, qs], o[0:D, :])
                    stg = ph2s.tile([1, 512], F32, tag="stg", name="stg")
                    nc.vector.tensor_copy(stg, o[D : D + 1, :])
                    nc.sync.dma_start(out=dtile[row : row + 1, qs], in_=stg)
                if stg_first:
                    for t2 in range(NT2):
                        qs = slice(t2 * 512, (t2 + 1) * 512)
                        nc.vector.tensor_copy(
                            oT[h // 2][po : po + D, qs], ovs[t2][0:D, :]
                        )

            def emit_pv_full(h):
                emit_pv_drain(h, emit_pv_mms(h))

            def emit_recip(dtile, r16tile):
                # 1/d as exp(-ln d) on ScalarE: the DVE reciprocal took ~8us
                # of DVE and serialized behind PV drains (ov-ring backlog ->
                # ~4us PE gap at the pair 3->4 boundary); ScalarE has idle
                # windows mid-stream and Ln/Exp share one act table.  fp16
                # output is plenty for a softmax denominator (5e-4 rel).
                nr = dtile.shape[0]
                with nc.allow_low_precision("softmax denom recip to fp16"):
                    nc.scalar.activation(
                        lnS[0:nr, :], dtile, mybir.ActivationFunctionType.Ln
                    )
                    nc.scalar.activation(
                        r16tile, lnS[0:nr, :],
                        mybir.ActivationFunctionType.Exp, scale=-1.0,
                    )

            def emit_r_chunk(r16tile, i, c, nr=4):
                ind_t = ind4_t
                ps = [ov_tile(), ov_tile()]
                for t2 in range(NT2):
                    nc.tensor.matmul(
                        ps[t2],
                        ind_t[0:nr, i * 128 : (i + 1) * 128],
                        r16tile[:, t2 * 512 : (t2 + 1) * 512],
                        start=True,
                        stop=True,
                    )
                rr = ph2s.tile([128, N], F16, tag="r16", name="r16", bufs=2)
                for t2 in range(NT2):
                    nc.vector.tensor_copy(rr[:, t2 * 512 : (t2 + 1) * 512], ps[t2])
                nc.vector.tensor_mul(oT[c], oT[c], rr)

            # ---------------- lead-in: just enough qk for ST(0,0) ---------
            for j, t2 in ((0, 0), (0, 1), (CC, 0)):
                emit_1a(j, t2)

            # ---------------- attention: global slot stream ---------------
            # STs are emitted two slots ahead of their slot's chain items so
            # a multi-us chain item never delays the next ACT; chain items
            # are kept fine (<= ~2us).
            def i1a(j, t2):
                return lambda: emit_1a(j, t2)

            def i1b(t8, nh):
                return lambda: emit_1b(t8, nh)

            chains = {p_: [] for p_ in range(NP)}
            chains[0] = [i1a(CC, 1), i1a(1, 0), i1a(1, 1), i1a(CC + 1, 0),
                         i1a(CC + 1, 1)]
            chains[0] += [i1b(t8, nh) for t8 in range(6) for nh in range(2)]
            chains[1] = [i1b(t8, nh) for t8 in (6, 7) for nh in range(2)]
            for p_ in (1, 2, 3, 4):
                for j in (p_ + 1, CC + p_ + 1):
                    for t2 in range(NT2):
                        chains[p_].append(i1a(j, t2))
            chains[1] += [*ipv(0),
                          *ipv(1)]
            chains[2] += [*ipv(2),
                          *ipv(3)]
            # the batched reciprocals are 8us of DVE each: schedule them
            # where the DVE queue has slack so they never delay a PV drain
            # (a delayed drain stalls the ov ring -> PV backlog -> the last
            # STs block in the PE FIFO -> ScalarE starves; that cascade cost
            # v4/v6 ~25us)
            chains[3] = ([lambda: emit_recip(dallA1, rec16A1)] + chains[3]
                         + [*ipv(4),
                            *ipv(5)])
            chains[4] += [*ipv(6),
                          *ipv(7),
                          lambda: emit_r_chunk(rec16A1, 0, 0),
                          lambda: emit_r_chunk(rec16A1, 1, 1),
                          lambda: emit_recip(dallA2, rec16A2)]
            # r_chunk(A2,*) moved to the tail: as pair-5 chain items they
            # formed a serial PE<->DVE chain through the 2-slot ov ring
            # (r2 -> rr copies -> r3 -> rr -> pv10 -> p11) that could not
            # fit in the last slots and pushed both trailing PVs past the
            # stream end at cold p-state.
            chains[5] = [
                lambda: pv_open(8), lambda: pv_close(8),
                lambda: pv_open(9), lambda: pv_close(9),
            ]

            slots = [(p_, kc) for p_ in range(NP) for kc in range(NT8)]
            slot_items = [[] for _ in slots]
            for p_ in range(NP - 1):
                items = chains[p_]
                done = 0
                for kc in range(NT8):
                    want = (len(items) * (kc + 1) + NT8 - 1) // NT8
                    while done < want:
                        slot_items[p_ * NT8 + kc].append(items[done])
                        done += 1

            # pair 5 is pinned by hand: every ov-ring user must be emitted
            # before pv10 opens (it holds both slots until the tail drain),
            # and pv10's kc MMs trail its ACTs by ~2 kc so the accumulation
            # hides under the last exps instead of running post-stream.
            # pv11 ALSO trails, accumulating into a 97th st-ring tile
            # (allocated only after all 96 ST allocations so it cannot
            # poison the ring) viewed as two [128,512] halves.
            ovs10 = []
            p11v = []

            def pv10_start():
                ovs10.extend(ov_tile() for _ in range(NT2))
                pv_mm_range(10, ovs10, range(4))

            def pv10_mm(kc):
                return lambda: pv_mm_range(10, ovs10, (kc,))

            def p11_start():
                t = st_tile()
                p11v.extend(t[:, t2 * 512 : (t2 + 1) * 512] for t2 in range(NT2))
                pv_mm_range(11, p11v, range(4))

            def p11_mm(kc):
                return lambda: pv_mm_range(11, p11v, (kc,))

            c5 = chains[5]
            for kc, items in enumerate((
                [c5[0]], [c5[1]], [c5[2]], [c5[3]],
                [p11_start],
                [pv10_start],
                [pv10_mm(4), pv10_mm(5), p11_mm(4), p11_mm(5)],
                [pv10_mm(6), p11_mm(6)],
            )):
                slot_items[5 * NT8 + kc].extend(items)

            LOOKAHEAD = 2
            for i in range(LOOKAHEAD):
                emit_st_pair(*slots[i])
            for i in range(len(slots)):
                if i + LOOKAHEAD < len(slots):
                    emit_st_pair(*slots[i + LOOKAHEAD])
                for item in slot_items[i]:
                    item()

            # ---------------- tail ----------------------------------------
            # pv10/pv11 already trailed the exp stream; only their kc=7
            # accumulations + drains remain.  The denominators were split
            # into [2,N] pairs so the 8/9 recip chain (ln+exp on ScalarE)
            # starts right after the last softmax EXP and head 10/11's only
            # waits on its own drain, not on a batched [4,N] assembly.  The
            # output projection prefills c=0..3 (then c=4 once oT4 is
            # normalized) for the first three fc chunks while the 10/11
            # recip chain resolves, so only the c=5 accumulations, biases
            # and the (ring-split) output DMAs remain at the very end.
            pv_mm_range(10, ovs10, (7,))
            emit_pv_drain(10, ovs10, stg_first=True)
            pv_mm_range(11, p11v, (7,))
            emit_pv_drain(11, p11v, stg_first=True)
            emit_recip(dallB89, rec16B89)
            emit_recip(dallB11, rec16B11)

            with tc.tile_pool(name="ph3o", bufs=2) as ph3o:

                def ph3_mms(p, fc, cs, start):
                    for c in cs:
                        for t2 in range(NT2):
                            nc.tensor.matmul(
                                p[:, t2 * 512 : (t2 + 1) * 512],
                                wo[c][:, fc * 128 : (fc + 1) * 128],
                                oT[c][:, t2 * 512 : (t2 + 1) * 512],
                                start=(start and c == cs[0]),
                                stop=(c == CC - 1),
                            )

                def ph3_finish(p, fc):
                    ph3_mms(p, fc, (5,), False)
                    ot = ph3o.tile([128, N], F32, tag="outsb", name="outsb")
                    # bias-add as Identity-activation on ScalarE (idle after
                    # the exp stream; `identity` shares the ln/exp act table
                    # so no table swap).  On DVE these 6 adds queued behind
                    # the drain/normalize copies and trailed the last matmul
                    # by ~5us.
                    nc.scalar.activation(
                        ot, p, mybir.ActivationFunctionType.Identity,
                        bias=bo_t[fc],
                    )
                    # one 512KB dma_start lands on ~2 DMA rings (~22.5GB/s
                    # each) and drained ~11us past the last compute; split
                    # across rings and both HWDGE queues instead (8-way for
                    # the last chunk, which bounds kernel end).
                    # fc<5 stores go through the Pool SWDGE queue (separate
                    # DGE unit) so the sync DGE is empty when the last
                    # chunk's 8-way split arrives — DGE generation (~640ns
                    # per dma_start, serial per queue) bounds the drain.
                    if fc == CC - 1:
                        nsp, eng = 8, nc.sync
                    else:
                        nsp, eng = 2, nc.gpsimd
                    step = 128 // nsp
                    for i in range(nsp):
                        eng.dma_start(
                            out=outT[fc * 128 + i * step : fc * 128 + (i + 1) * step, :],
                            in_=ot[i * step : (i + 1) * step, :],
                        )

                ph3_ps = []
                for fc in range(3):
                    p = st_tile()
                    ph3_mms(p, fc, (0, 1), True)
                    ph3_ps.append(p)
                emit_r_chunk(rec16A2, 0, 2)
                emit_r_chunk(rec16A2, 1, 3)
                for fc in range(3):
                    ph3_mms(ph3_ps[fc], fc, (2, 3), False)
                emit_r_chunk(rec16B89, 0, 4, nr=2)
                for fc in range(3):
                    ph3_mms(ph3_ps[fc], fc, (4,), False)
                emit_r_chunk(rec16B11, 0, 5, nr=2)
                for fc in range(3):
                    ph3_finish(ph3_ps[fc], fc)
                for fc in range(3, CC):
                    p = st_tile()
                    ph3_mms(p, fc, (0, 1, 2, 3, 4), True)
                    ph3_finish(p, fc)

    if os.environ.get("KERNEL_HOIST"):
        # measured neutral-to-negative on HW (fusion count unchanged, PE
        # sequencer +25us from the extra NoOps); keep for experiments only
        _hoist_st_waits(nc)
    if split:
        _split_multiwaits(nc)
    return nc


_NC = None


def _get_nc():
    global _NC
    if _NC is None:
        _NC = _build()
    return _NC


def kernel(x, w_qkv, w_out, b_out):
    x = np.asarray(x, dtype=np.float32)
    w_qkv = np.asarray(w_qkv, dtype=np.float32)
    w_out = np.asarray(w_out, dtype=np.float32)
    b_out = np.asarray(b_out, dtype=np.float32)

    wqkT = np.ascontiguousarray(w_qkv[: 2 * C].T.astype(np.float16))
    wvT = np.ascontiguousarray(w_qkv[2 * C :].T.astype(np.float16))
    woT = np.ascontiguousarray(w_out.T.astype(np.float16))
    bo = np.ascontiguousarray(b_out.reshape(C, 1))
    ind4 = np.zeros((4, 2 * 128), dtype=np.float16)
    for c in range(2):
        ind4[2 * c, c * 128 : c * 128 + D] = 1.0
        ind4[2 * c + 1, c * 128 + D : (c + 1) * 128] = 1.0

    in_maps = [
        {
            "xT": np.ascontiguousarray(x[b].T.astype(np.float16)),
            "wqkT": wqkT,
            "wvT": wvT,
            "woT": woT,
            "bo": bo,
            "ind4": ind4,
        }
        for b in range(B)
    ]

    nc = _get_nc()
    trace = bool(os.environ.get("KERNEL_TRACE"))
    res = run_bass_kernel_spmd(nc, in_maps, list(range(_N_CORES)), trace=trace)
    if trace:
        print(f"HW exec time: {res.exec_time_ns} ns")
        if res.instructions_and_trace is not None:
            print(f"trace: {res.instructions_and_trace[1]}")

    out = np.empty((B, N, C), dtype=np.float32)
    for b in range(B):
        out[b] = res.results[b]["outT"].T
    return out



# revision 49
# speedup vs baseline: 1.0344x; 1.0344x over previous
"""Multi-head attention (B=8, N=1024, C=768, H=12) on 8 TRN2 NeuronCores.

Data-parallel over batch: core b computes batch element b end-to-end.

v3: software-pipelined stream over head PAIRS (row-packed 64-contraction
STs, exp(S*scale-5) on ScalarE, PV with the ones-column denominator
trick), plus:

  - ALL softmax-denominator reciprocals run on ScalarE as exp(-ln d)
    (Ln/Exp share one act table).  The old batched DVE reciprocals were
    ~8us each and serialized behind PV drains in the DVE queue, stalling
    the 2-slot ov PSUM ring and opening ~4us PE gaps.
  - every DMA issues from the SP (sync) HWDGE queue only: issuing from
    the Activation HWDGE queue in parallel throttled the whole chip ~9%
    (warm 512-row matmul 215->235ns) - measured, reproducible.
  - lead-in DMA order is consumption order (wqks j0, xr t2=0, xr t2=1,
    wqks jCC, ...), and a no-dep dummy Exp pulls the act-table load to
    ~9us so the first real EXP isn't stalled behind it.
  - pv10 trails the exp stream in the ov ring; pv11 trails in a 97th
    st-ring tile (allocated after all 96 ST allocations - earlier would
    poison the ring: a later ST tile would WAR-wait on pv11's late
    drain).  Their denominators land in split [2,N] tiles so the 8/9
    recip chain starts right at the last softmax EXP and 10/11's only
    waits on its own drain.
  - the output projection prefills c=0..1, then normalizes oT2/oT3
    (r_chunks moved out of the pair-5 chain - as chain items they formed
    a serial PE<->DVE chain through the ov ring that spilled past the
    stream end at cold p-state), then c=2..4 as recips resolve; only the
    c=5 accumulations, biases and ring-split output DMAs remain at the
    end.  Output stores are split 4/8-way across DMA rings (one 512KB
    dma_start lands on ~2 rings at ~22.5GB/s and drained ~11us past the
    last compute).

PSUM budget: ST ring 3 x [128,1024] (6 banks) + work ring 2 x [128,512]
(2 banks) = 8 banks.  Host side casts x/weights to fp16 and transposes
the fp32 outT result back.
"""

import os

import numpy as np

import concourse.bass as bass
import concourse.tile as tile
from concourse import mybir
from concourse.bass_utils import run_bass_kernel_spmd

B, N, C = 8, 1024, 768
H, D = 12, 64
NP = H // 2            # head pairs
SCALE = D ** -0.5
EXP_BIAS = -5.0
CC = C // 128          # 6 contraction chunks
NT2 = N // 512         # 2 free-dim chunks of 512
NT8 = N // 128         # 8 partition chunks of 128
F32 = mybir.dt.float32
F16 = mybir.dt.float16

_N_CORES = 8


def _split_multiwaits(nc, max_waits: int = 1):
    """The pinned walrus codegen supports one embedded sync-wait per engine
    instruction (single EVENTS slot in the TPB ISA).  Tile's tail drain /
    barriers accumulate several; hoist all-but-one wait onto same-engine
    NoOps placed immediately before the instruction (waits AND, so order is
    irrelevant)."""
    n_split = 0
    for f in nc.m.functions:
        for blk in f.blocks:
            insts = blk.instructions
            if not any(
                ins.sync_info is not None and len(ins.sync_info.on_wait) > max_waits
                for ins in insts
            ):
                continue
            new_list = []
            for ins in insts:
                si = ins.sync_info
                if si is not None and len(si.on_wait) > max_waits:
                    waits = list(si.on_wait)
                    hoist, keep = waits[:-max_waits], waits[-max_waits:]
                    for w in hoist:
                        nop = mybir.InstNoOp(name=nc.get_next_instruction_name())
                        nop.engine = ins.engine
                        nop.sync_info = mybir.SyncInfo(on_wait=[w], on_update=[])
                        new_list.append(nop)
                        n_split += 1
                    ins.sync_info = mybir.SyncInfo(
                        on_wait=keep, on_update=list(si.on_update)
                    )
                new_list.append(ins)
            blk.instructions = new_list
    return n_split


def _hoist_st_waits(nc):
    """ST (64-contraction) matmuls packed at tile_position (0,0)/(64,0) can
    execute CONCURRENTLY on the PE's two quadrant rows, but only when the
    trailing matmul reaches the array with no unsatisfied sync-wait (the
    trace shows fused trailers retiring in ~3ns; any wait serializes them at
    full cost).  The st-ring WAR waits land on the first writer of each
    recycled PSUM tile, i.e. in the middle of the 4-matmul ST group.  Hoist
    every wait inside a run of consecutive ST units (Ldweights[64p] +
    NoOps + Matmult) onto fresh PE NoOps placed just before the run, so the
    run's matmuls issue back-to-back and pair up.  Waits only move EARLIER
    on the same engine queue, which is always safe (their producers never
    depend on the skipped instructions)."""
    n_runs = 0
    for f in nc.m.functions:
        for blk in f.blocks:
            insts = blk.instructions
            # collect (block-pos) of PE instructions
            pe_pos = [i for i, ins in enumerate(insts)
                      if ins.engine == mybir.EngineType.PE]

            def is_st_ldw(ins):
                if type(ins).__name__ != "InstLdweights":
                    return False
                try:
                    ap = list(ins.ins[0].ap)
                    return ap[0][1] == 64
                except Exception:
                    return False

            # walk the PE subsequence, grouping ST units into runs.  A run
            # window may cover at most 2 distinct output PSUM tiles (= one
            # ST pair): with the 3-deep st ring, a longer window can contain
            # the producer matmuls of the very ACT a hoisted WAR wait blocks
            # on (ring wrap) -> PE/ScalarE deadlock.  Windows of <=2 newest
            # allocations can never include the tile an in-window WAR wait's
            # ACT reads (that tile is >=3 allocations older).
            def mm_out_ref(ins):
                try:
                    return ins.outs[0].memref
                except Exception:
                    return None

            runs = []  # list of lists of block positions
            cur = []
            cur_tiles = []

            def flush():
                nonlocal cur, cur_tiles
                if len(cur) >= 2:
                    runs.append(cur)
                cur = []
                cur_tiles = []

            k = 0
            while k < len(pe_pos):
                ins = insts[pe_pos[k]]
                if is_st_ldw(ins):
                    unit = [pe_pos[k]]
                    k += 1
                    # absorb NoOps then the matmult
                    while k < len(pe_pos) and type(insts[pe_pos[k]]).__name__ == "InstNoOp":
                        unit.append(pe_pos[k])
                        k += 1
                    if k < len(pe_pos) and type(insts[pe_pos[k]]).__name__ == "InstMatmult":
                        mm = insts[pe_pos[k]]
                        unit.append(pe_pos[k])
                        k += 1
                        ref = mm_out_ref(mm)
                        if ref not in cur_tiles and len(cur_tiles) >= 2:
                            flush()
                        if ref not in cur_tiles:
                            cur_tiles.append(ref)
                        cur.append(unit)
                        continue
                    flush()
                else:
                    flush()
                    k += 1
            flush()

            if not runs:
                continue
            # hoist waits: build new instruction list in one pass
            prepend = {}  # head block-pos -> list of waits
            for run in runs:
                waits = []
                for unit in run:
                    for pos in unit:
                        si = insts[pos].sync_info
                        if si is not None and si.on_wait:
                            waits.extend(si.on_wait)
                            insts[pos].sync_info = mybir.SyncInfo(
                                on_wait=[], on_update=list(si.on_update)
                            )
                if waits:
                    prepend[run[0][0]] = waits
                    n_runs += 1
            if not prepend:
                continue
            new_list = []
            for i, ins in enumerate(insts):
                if i in prepend:
                    for w in prepend[i]:
                        nop = mybir.InstNoOp(name=nc.get_next_instruction_name())
                        nop.engine = mybir.EngineType.PE
                        nop.sync_info = mybir.SyncInfo(on_wait=[w], on_update=[])
                        new_list.append(nop)
                new_list.append(ins)
            blk.instructions = new_list
    return n_runs


def _dedup_st_ldw(nc):
    """Drop redundant ST Ldweights: within a burst the same 64-row
    stationary (quadrant row 0-63 or 64-127) is re-loaded for the second
    free-dim half although no other load touched that quadrant.  Replacing
    the reload with a NoOp (keeping its sync_info so producer/consumer sems
    still fire) removes exposed load time and leaves matmuls back-to-back,
    which the PE can chain.  Any non-64-partition Ldweights clobbers both
    quadrants conservatively."""
    n_drop = 0
    for f in nc.m.functions:
        for blk in f.blocks:
            last_q = {0: None, 64: None}
            insts = blk.instructions
            for i, ins in enumerate(insts):
                if ins.engine != mybir.EngineType.PE:
                    continue
                if type(ins).__name__ != "InstLdweights":
                    continue
                try:
                    pap = ins.ins[0]
                    ap = list(pap.ap)
                    pstride, pnum = ap[0]
                    off = pap.offset
                    memref = pap.memref
                except Exception:
                    last_q = {0: None, 64: None}
                    continue
                if pnum != 64:
                    last_q = {0: None, 64: None}
                    continue
                base = off // pstride if pstride else 0
                q = 0 if base < 64 else 64
                key = (memref, off)
                if last_q[q] == key:
                    nop = mybir.InstNoOp(name=nc.get_next_instruction_name())
                    nop.engine = mybir.EngineType.PE
                    nop.sync_info = ins.sync_info
                    insts[i] = nop
                    n_drop += 1
                else:
                    last_q[q] = key
    return n_drop


def _build(split: bool = True):
    nc = bass.Bass()
    xT = nc.declare_dram_parameter("xT", [C, N], F16, isOutput=False)
    wqkT = nc.declare_dram_parameter("wqkT", [C, 2 * C], F16, isOutput=False)
    wvT = nc.declare_dram_parameter("wvT", [C, C], F16, isOutput=False)
    woT = nc.declare_dram_parameter("woT", [C, C], F16, isOutput=False)
    bo = nc.declare_dram_parameter("bo", [C, 1], F32, isOutput=False)
    ind4_d = nc.declare_dram_parameter("ind4", [4, 2 * 128], F16, isOutput=False)
    outT = nc.declare_dram_parameter("outT", [C, N], F32, isOutput=True)

    with tile.TileContext(nc) as tc:
        with (
            tc.tile_pool(name="sb", bufs=1) as sb,
            tc.tile_pool(name="ph2", bufs=33) as ph2,
            tc.tile_pool(name="ph2s", bufs=3) as ph2s,
            tc.tile_pool(name="psum", bufs=1, space="PSUM") as psum,
        ):
            qkT = [
                sb.tile([128, N], F16, tag=f"qkT{j}", name=f"qkT{j}")
                for j in range(2 * CC)
            ]
            v_sb = [
                sb.tile([128, H * (D + 1)], F16, tag=f"v{t}", name=f"v{t}")
                for t in range(NT8)
            ]
            oT = [sb.tile([128, N], F16, tag=f"oT{c}", name=f"oT{c}") for c in range(CC)]
            bo_t = [sb.tile([128, 1], F32, tag=f"bo{c}", name=f"bo{c}") for c in range(CC)]
            xr = [sb.tile([128, N], F16, tag=f"xr{c}", name=f"xr{c}") for c in range(CC)]
            wqk = [
                sb.tile([128, 2 * C], F16, tag=f"wqk{c}", name=f"wqk{c}")
                for c in range(CC)
            ]
            wv = [sb.tile([128, C], F16, tag=f"wv{c}", name=f"wv{c}") for c in range(CC)]
            wo = [sb.tile([128, C], F16, tag=f"wo{c}", name=f"wo{c}") for c in range(CC)]

            # DMA lead-in: the first ST needs xr[:, t2=0] + the wqks j=0/CC
            # duplicated slices.  Issue latency on one queue (~565ns per
            # dma_start on the SP sequencer) dominated the old 17us lead-in,
            # so spread the critical transfers across BOTH HWDGE queues:
            # sync takes x, scalar takes the small weight slices (the Act
            # engine is idle until the first EXP at ~26us).
            # NOTE: all DMAs must issue from the SP (sync) HWDGE queue.  Using
            # the Activation engine's HWDGE queue in parallel looked free on
            # paper but throttled the whole chip ~9% (every engine's warm
            # cycle time inflated 215->235ns/512rows) — measured +8.5us.
            wqks = {}
            _dma2 = nc.sync

            def wqks_dma(j, c, eng=None):
                js = slice(j * 128, (j + 1) * 128)
                sl = slice(c * 128, (c + 1) * 128)
                t = sb.tile([128, 128], F16, tag=f"wqks{j}_{c}", name=f"wqks{j}_{c}")
                (eng or nc.sync).dma_start(out=t, in_=wqkT[sl, js])
                wqks[(j, c)] = t

            # j0 slices via the Pool SWDGE queue: takes 6 issues off the
            # sync DGE's critical lead-in so the xr transfers start ~3.4us
            # earlier (sync DGE gen is ~565ns per dma_start, serial).
            for c in range(CC):
                wqks_dma(0, c, eng=nc.gpsimd)
            for c in range(CC):
                sl = slice(c * 128, (c + 1) * 128)
                nc.sync.dma_start(out=xr[c][:, 0:512], in_=xT[sl, 0:512])
            # xr t2=1 before wqks jCC: the lead-in order is 1a(0,0) [xr
            # t2=0 + wqks j0], 1a(0,1) [xr t2=1], 1a(CC,0) [wqks jCC]
            for c in range(CC):
                sl = slice(c * 128, (c + 1) * 128)
                nc.sync.dma_start(out=xr[c][:, 512:1024], in_=xT[sl, 512:1024])
            for c in range(CC):
                wqks_dma(CC, c)
            # pull the Exp act-table load off the first-EXP critical path:
            # a no-dep dummy exp right after the critical weight slices makes
            # the fixpoint table-load pass plant the load at ~9us instead of
            # right before the first real EXP.
            tbl = sb.tile([1, 2], F32, tag="tbl")
            nc.vector.memset(tbl[:, 0:1], 0.0)
            nc.scalar.activation(
                tbl[:, 1:2], tbl[:, 0:1], mybir.ActivationFunctionType.Exp
            )
            for j in (1, CC + 1):
                for c in range(CC):
                    js = slice(j * 128, (j + 1) * 128)
                    sl = slice(c * 128, (c + 1) * 128)
                    t = sb.tile([128, 128], F16, tag=f"wqks{j}_{c}", name=f"wqks{j}_{c}")
                    nc.sync.dma_start(out=t, in_=wqkT[sl, js])
                    wqks[(j, c)] = t
            for c in range(CC):
                sl = slice(c * 128, (c + 1) * 128)
                nc.sync.dma_start(out=wv[c], in_=wvT[sl, :])
            for c in range(CC):
                sl = slice(c * 128, (c + 1) * 128)
                nc.sync.dma_start(out=wqk[c], in_=wqkT[sl, :])
            for c in range(CC):
                sl = slice(c * 128, (c + 1) * 128)
                nc.sync.dma_start(out=wo[c], in_=woT[sl, :])
                nc.sync.dma_start(out=bo_t[c], in_=bo[sl, :])

            ones12 = sb.tile([128, H], F16, tag="ones12")
            nc.vector.memset(ones12, 1.0)
            ebias = sb.tile([128, 1], F32, tag="ebias")
            nc.vector.memset(ebias, EXP_BIAS)

            dallA1 = sb.tile([4, N], F32, tag="dallA1")
            dallA2 = sb.tile([4, N], F32, tag="dallA2")
            # tail denominators alias rows 0-1 of the A tiles: lifetimes are
            # disjoint (lnA1/lnA2 consume the A rows around pair 3-4, the
            # B-group drains write at pair 5 / tail; Tile's WAR edges keep
            # it safe), freeing 8KB of columns for the exps pool (29->33:
            # the live window of exp tiles is ~2 pairs = 32+, so the ACT
            # stream was throttled by pool WAR).
            dallB89 = dallA1[0:2, :]
            dallB11 = dallA2[0:2, :]
            lnS = sb.tile([4, N], F32, tag="lnS")  # shared ln scratch (A+B)
            rec16A1 = sb.tile([4, N], F16, tag="rec16A1")
            rec16A2 = sb.tile([4, N], F16, tag="rec16A2")
            rec16B89 = sb.tile([2, N], F16, tag="rec16B89")
            rec16B11 = sb.tile([2, N], F16, tag="rec16B11")
            ind4_t = sb.tile([4, 2 * 128], F16, tag="ind4")
            nc.sync.dma_start(out=ind4_t, in_=ind4_d[:, :])

            def st_tile():
                return psum.tile([128, N], F32, tag="st", name="st", bufs=3)

            def ov_tile():
                return psum.tile([128, 512], F32, tag="ov", name="ov", bufs=2)

            # ---------------- emission helpers ----------------
            def emit_1a(j, t2):
                # qkT[j][:, t2-half] = sum_c wqk[c][:, j-block].T @ xr[c][:, t2]
                p = ov_tile()
                for c in range(CC):
                    w = (
                        wqks[(j, c)]
                        if (j, c) in wqks
                        else wqk[c][:, j * 128 : (j + 1) * 128]
                    )
                    nc.tensor.matmul(
                        p,
                        w,
                        xr[c][:, t2 * 512 : (t2 + 1) * 512],
                        start=(c == 0),
                        stop=(c == CC - 1),
                    )
                nc.vector.tensor_copy(qkT[j][:, t2 * 512 : (t2 + 1) * 512], p)

            def emit_1b(t8, nh):
                # v[t8][:, h*65:h*65+64] = sum_c xr[c][:, t8].T @ wv[c]
                p = ov_tile()
                for c in range(CC):
                    nc.tensor.matmul(
                        p[:, 0:384],
                        xr[c][:, t8 * 128 : (t8 + 1) * 128],
                        wv[c][:, nh * 384 : (nh + 1) * 384],
                        start=(c == 0),
                        stop=(c == CC - 1),
                    )
                v_view = v_sb[t8].rearrange("p (h e) -> p h e", e=D + 1)
                nc.vector.tensor_copy(
                    v_view[:, nh * 6 : (nh + 1) * 6, 0:D],
                    p[:, 0:384].rearrange("p (h d) -> p h d", d=D),
                )
                if nh == 1:
                    nc.vector.tensor_copy(
                        v_view[:, :, D : D + 1], ones12.unsqueeze(2)
                    )

            exps = {}

            def emit_st_pair(p_, kc):
                # row-packed: even head in PE rows 0-63, odd head in 64-127
                se = st_tile()
                so = st_tile()
                kb = slice(kc * 128, (kc + 1) * 128)
                for t2 in range(NT2):
                    sl = slice(t2 * 512, (t2 + 1) * 512)
                    nc.tensor.matmul(
                        se[:, sl], qkT[CC + p_][0:D, kb], qkT[p_][0:D, sl],
                        start=True, stop=True,
                    )
                    nc.tensor.matmul(
                        so[:, sl], qkT[CC + p_][D:128, kb], qkT[p_][D:128, sl],
                        start=True, stop=True,
                    )
                ee = ph2.tile([128, N], F16, tag="exps", name="exps")
                eo = ph2.tile([128, N], F16, tag="exps", name="exps")
                nc.scalar.activation(
                    ee, se, mybir.ActivationFunctionType.Exp, scale=SCALE, bias=ebias
                )
                nc.scalar.activation(
                    eo, so, mybir.ActivationFunctionType.Exp, scale=SCALE, bias=ebias
                )
                exps[(2 * p_, kc)] = ee
                exps[(2 * p_ + 1, kc)] = eo

            pv_state = {}

            def pv_mm_range(h, ovs, kcs):
                for kc in kcs:
                    e = exps.pop((h, kc))
                    for t2 in range(NT2):
                        nc.tensor.matmul(
                            ovs[t2][0 : D + 1, :],
                            v_sb[kc][:, h * (D + 1) : (h + 1) * (D + 1)],
                            e[:, t2 * 512 : (t2 + 1) * 512],
                            start=(kc == 0),
                            stop=(kc == NT8 - 1),
                        )

            def emit_pv_mms(h):
                ovs = [ov_tile() for _ in range(NT2)]
                pv_mm_range(h, ovs, range(NT8))
                return ovs

            def pv_q(h, q):
                # quarter-granularity PV chain items (4 MMs each) so a PV
                # burst never exceeds one slot's PE budget; q=0 allocates,
                # q=3 drains.  Parts of one head must stay adjacent in the
                # chain list (ov-ring WAR deadlock otherwise).
                if q == 0:
                    ovs = [ov_tile() for _ in range(NT2)]
                    pv_state[h] = ovs
                    pv_mm_range(h, ovs, range(2))
                else:
                    pv_mm_range(h, pv_state[h], range(2 * q, 2 * q + 2))
                    if q == 3:
                        emit_pv_drain(h, pv_state.pop(h))

            def ipv(h):
                return [lambda q=q: pv_q(h, q) for q in range(4)]

            def pv_open(h):
                # first half of the PV accumulation; holds both ov-ring
                # slots until pv_close — no other ov-allocating item may be
                # emitted in between (ring WAR would deadlock the PE FIFO)
                ovs = [ov_tile() for _ in range(NT2)]
                pv_state[h] = ovs
                pv_mm_range(h, ovs, range(NT8 // 2))

            def pv_close(h):
                ovs = pv_state.pop(h)
                pv_mm_range(h, ovs, range(NT8 // 2, NT8))
                emit_pv_drain(h, ovs)

            def emit_pv_drain(h, ovs, stg_first=False):
                po = (h % 2) * 64
                dtile, row = (
                    (dallA1, h) if h < 4
                    else (dallA2, h - 4) if h < 8
                    else (dallB89, h - 8) if h < 10
                    else (dallB11, h - 10)
                )
                for t2 in range(NT2):
                    o = ovs[t2]
                    qs = slice(t2 * 512, (t2 + 1) * 512)
                    if not stg_first:
                        nc.vector.tensor_copy(oT[h // 2][po : po + D, qs], o[0:D, :])
                    stg = ph2s.tile([1, 512], F32, tag="stg", name="stg")
                    nc.vector.tensor_copy(stg, o[D : D + 1, :])
                    nc.sync.dma_start(out=dtile[row : row + 1, qs], in_=stg)
                if stg_first:
                    for t2 in range(NT2):
                        qs = slice(t2 * 512, (t2 + 1) * 512)
                        nc.vector.tensor_copy(
                            oT[h // 2][po : po + D, qs], ovs[t2][0:D, :]
                        )

            def emit_pv_full(h):
                emit_pv_drain(h, emit_pv_mms(h))

            def emit_recip(dtile, r16tile):
                # 1/d as exp(-ln d) on ScalarE: the DVE reciprocal took ~8us
                # of DVE and serialized behind PV drains (ov-ring backlog ->
                # ~4us PE gap at the pair 3->4 boundary); ScalarE has idle
                # windows mid-stream and Ln/Exp share one act table.  fp16
                # output is plenty for a softmax denominator (5e-4 rel).
                nr = dtile.shape[0]
                with nc.allow_low_precision("softmax denom recip to fp16"):
                    nc.scalar.activation(
                        lnS[0:nr, :], dtile, mybir.ActivationFunctionType.Ln
                    )
                    nc.scalar.activation(
                        r16tile, lnS[0:nr, :],
                        mybir.ActivationFunctionType.Exp, scale=-1.0,
                    )

            def emit_r_chunk(r16tile, i, c, nr=4):
                ind_t = ind4_t
                ps = [ov_tile(), ov_tile()]
                for t2 in range(NT2):
                    nc.tensor.matmul(
                        ps[t2],
                        ind_t[0:nr, i * 128 : (i + 1) * 128],
                        r16tile[:, t2 * 512 : (t2 + 1) * 512],
                        start=True,
                        stop=True,
                    )
                rr = ph2s.tile([128, N], F16, tag="r16", name="r16", bufs=2)
                for t2 in range(NT2):
                    nc.vector.tensor_copy(rr[:, t2 * 512 : (t2 + 1) * 512], ps[t2])
                nc.vector.tensor_mul(oT[c], oT[c], rr)

            # ---------------- lead-in: just enough qk for ST(0,0) ---------
            for j, t2 in ((0, 0), (0, 1), (CC, 0)):
                emit_1a(j, t2)

            # ---------------- attention: global slot stream ---------------
            # STs are emitted two slots ahead of their slot's chain items so
            # a multi-us chain item never delays the next ACT; chain items
            # are kept fine (<= ~2us).
            def i1a(j, t2):
                return lambda: emit_1a(j, t2)

            def i1b(t8, nh):
                return lambda: emit_1b(t8, nh)

            chains = {p_: [] for p_ in range(NP)}
            chains[0] = [i1a(CC, 1), i1a(1, 0), i1a(1, 1), i1a(CC + 1, 0),
                         i1a(CC + 1, 1)]
            chains[0] += [i1b(t8, nh) for t8 in range(6) for nh in range(2)]
            chains[1] = [i1b(t8, nh) for t8 in (6, 7) for nh in range(2)]
            for p_ in (1, 2, 3, 4):
                for j in (p_ + 1, CC + p_ + 1):
                    for t2 in range(NT2):
                        chains[p_].append(i1a(j, t2))
            chains[1] += [*ipv(0),
                          *ipv(1)]
            chains[2] += [*ipv(2),
                          *ipv(3)]
            # the batched reciprocals are 8us of DVE each: schedule them
            # where the DVE queue has slack so they never delay a PV drain
            # (a delayed drain stalls the ov ring -> PV backlog -> the last
            # STs block in the PE FIFO -> ScalarE starves; that cascade cost
            # v4/v6 ~25us)
            chains[3] = ([lambda: emit_recip(dallA1, rec16A1)] + chains[3]
                         + [*ipv(4),
                            *ipv(5)])
            chains[4] += [*ipv(6),
                          *ipv(7),
                          lambda: emit_r_chunk(rec16A1, 0, 0),
                          lambda: emit_r_chunk(rec16A1, 1, 1),
                          lambda: emit_recip(dallA2, rec16A2)]
            # r_chunk(A2,*) moved to the tail: as pair-5 chain items they
            # formed a serial PE<->DVE chain through the 2-slot ov ring
            # (r2 -> rr copies -> r3 -> rr -> pv10 -> p11) that could not
            # fit in the last slots and pushed both trailing PVs past the
            # stream end at cold p-state.
            chains[5] = [
                lambda: pv_open(8), lambda: pv_close(8),
                lambda: pv_open(9), lambda: pv_close(9),
            ]

            slots = [(p_, kc) for p_ in range(NP) for kc in range(NT8)]
            slot_items = [[] for _ in slots]
            for p_ in range(NP - 1):
                items = chains[p_]
                done = 0
                for kc in range(NT8):
                    want = (len(items) * (kc + 1) + NT8 - 1) // NT8
                    while done < want:
                        slot_items[p_ * NT8 + kc].append(items[done])
                        done += 1

            # pair 5 is pinned by hand: every ov-ring user must be emitted
            # before pv10 opens (it holds both slots until the tail drain),
            # and pv10's kc MMs trail its ACTs by ~2 kc so the accumulation
            # hides under the last exps instead of running post-stream.
            # pv11 ALSO trails, accumulating into a 97th st-ring tile
            # (allocated only after all 96 ST allocations so it cannot
            # poison the ring) viewed as two [128,512] halves.
            ovs10 = []
            p11v = []

            def pv10_start():
                ovs10.extend(ov_tile() for _ in range(NT2))
                pv_mm_range(10, ovs10, range(4))

            def pv10_mm(kc):
                return lambda: pv_mm_range(10, ovs10, (kc,))

            def p11_start():
                t = st_tile()
                p11v.extend(t[:, t2 * 512 : (t2 + 1) * 512] for t2 in range(NT2))
                pv_mm_range(11, p11v, range(4))

            def p11_mm(kc):
                return lambda: pv_mm_range(11, p11v, (kc,))

            c5 = chains[5]
            for kc, items in enumerate((
                [c5[0]], [c5[1]], [c5[2]], [c5[3]],
                [p11_start],
                [pv10_start],
                [pv10_mm(4), pv10_mm(5), p11_mm(4), p11_mm(5)],
                [pv10_mm(6), p11_mm(6)],
            )):
                slot_items[5 * NT8 + kc].extend(items)

            LOOKAHEAD = 2
            for i in range(LOOKAHEAD):
                emit_st_pair(*slots[i])
            for i in range(len(slots)):
                if i + LOOKAHEAD < len(slots):
                    emit_st_pair(*slots[i + LOOKAHEAD])
                for item in slot_items[i]:
                    item()

            # ---------------- tail ----------------------------------------
            # pv10/pv11 already trailed the exp stream; only their kc=7
            # accumulations + drains remain.  The denominators were split
            # into [2,N] pairs so the 8/9 recip chain (ln+exp on ScalarE)
            # starts right after the last softmax EXP and head 10/11's only
            # waits on its own drain, not on a batched [4,N] assembly.  The
            # output projection prefills c=0..3 (then c=4 once oT4 is
            # normalized) for the first three fc chunks while the 10/11
            # recip chain resolves, so only the c=5 accumulations, biases
            # and the (ring-split) output DMAs remain at the very end.
            pv_mm_range(10, ovs10, (7,))
            emit_pv_drain(10, ovs10, stg_first=True)
            pv_mm_range(11, p11v, (7,))
            emit_pv_drain(11, p11v, stg_first=True)
            emit_recip(dallB89, rec16B89)
            emit_recip(dallB11, rec16B11)

            with tc.tile_pool(name="ph3o", bufs=2) as ph3o:

                def ph3_mms(p, fc, cs, start):
                    for c in cs:
                        for t2 in range(NT2):
                            nc.tensor.matmul(
                                p[:, t2 * 512 : (t2 + 1) * 512],
                                wo[c][:, fc * 128 : (fc + 1) * 128],
                                oT[c][:, t2 * 512 : (t2 + 1) * 512],
                                start=(start and c == cs[0]),
                                stop=(c == CC - 1),
                            )

                def ph3_finish(p, fc):
                    ph3_mms(p, fc, (5,), False)
                    ot = ph3o.tile([128, N], F32, tag="outsb", name="outsb")
                    # bias-add as Identity-activation on ScalarE (idle after
                    # the exp stream; `identity` shares the ln/exp act table
                    # so no table swap).  On DVE these 6 adds queued behind
                    # the drain/normalize copies and trailed the last matmul
                    # by ~5us.
                    nc.scalar.activation(
                        ot, p, mybir.ActivationFunctionType.Identity,
                        bias=bo_t[fc],
                    )
                    # one 512KB dma_start lands on ~2 DMA rings (~22.5GB/s
                    # each) and drained ~11us past the last compute; split
                    # across rings and both HWDGE queues instead (8-way for
                    # the last chunk, which bounds kernel end).
                    # fc<5 stores go through the Pool SWDGE queue (separate
                    # DGE unit) so the sync DGE is empty when the last
                    # chunk's 8-way split arrives — DGE generation (~640ns
                    # per dma_start, serial per queue) bounds the drain.
                    if fc == CC - 1:
                        nsp, eng = 8, nc.sync
                    else:
                        nsp, eng = 2, nc.gpsimd
                    step = 128 // nsp
                    for i in range(nsp):
                        eng.dma_start(
                            out=outT[fc * 128 + i * step : fc * 128 + (i + 1) * step, :],
                            in_=ot[i * step : (i + 1) * step, :],
                        )

                ph3_ps = []
                for fc in range(3):
                    p = st_tile()
                    ph3_mms(p, fc, (0, 1), True)
                    ph3_ps.append(p)
                emit_r_chunk(rec16A2, 0, 2)
                emit_r_chunk(rec16A2, 1, 3)
                for fc in range(3):
                    ph3_mms(ph3_ps[fc], fc, (2, 3), False)
                emit_r_chunk(rec16B89, 0, 4, nr=2)
                for fc in range(3):
                    ph3_mms(ph3_ps[fc], fc, (4,), False)
                emit_r_chunk(rec16B11, 0, 5, nr=2)
                for fc in range(3):
                    ph3_finish(ph3_ps[fc], fc)
                for fc in range(3, CC):
                    p = st_tile()
                    ph3_mms(p, fc, (0, 1, 2, 3, 4), True)
                    ph3_finish(p, fc)

    if os.environ.get("KERNEL_HOIST"):
        # measured neutral-to-negative on HW (fusion count unchanged, PE
        # sequencer +25us from the extra NoOps); keep for experiments only
        _hoist_st_waits(nc)
    if split:
        _split_multiwaits(nc)
    return nc


_NC = None


def _get_nc():
    global _NC
    if _NC is None:
        _NC = _build()
    return _NC


def kernel(x, w_qkv, w_out, b_out):
    x = np.asarray(x, dtype=np.float32)
    w_qkv = np.asarray(w_qkv, dtype=np.float32)
    w_out = np.asarray(w_out, dtype=np.float32)
    b_out = np.asarray(b_out, dtype=np.float32)

    wqkT = np.ascontiguousarray(w_qkv[: 2 * C].T.astype(np.float16))
    wvT = np.ascontiguousarray(w_qkv[2 * C :].T.astype(np.float16))
    woT = np.ascontiguousarray(w_out.T.astype(np.float16))
    bo = np.ascontiguousarray(b_out.reshape(C, 1))
    ind4 = np.zeros((4, 2 * 128), dtype=np.float16)
    for c in range(2):
        ind4[2 * c, c * 128 : c * 128 + D] = 1.0
        ind4[2 * c + 1, c * 128 + D : (c + 1) * 128] = 1.0

    in_maps = [
        {
            "xT": np.ascontiguousarray(x[b].T.astype(np.float16)),
            "wqkT": wqkT,
            "wvT": wvT,
            "woT": woT,
            "bo": bo,
            "ind4": ind4,
        }
        for b in range(B)
    ]

    nc = _get_nc()
    trace = bool(os.environ.get("KERNEL_TRACE"))
    res = run_bass_kernel_spmd(nc, in_maps, list(range(_N_CORES)), trace=trace)
    if trace:
        print(f"HW exec time: {res.exec_time_ns} ns")
        if res.instructions_and_trace is not None:
            print(f"trace: {res.instructions_and_trace[1]}")

    out = np.empty((B, N, C), dtype=np.float32)
    for b in range(B):
        out[b] = res.results[b]["outT"].T
    return out

